# revision 24
# baseline (speedup 1.0000x reference)
"""Distributed Trainium2 kernel for nn_Attention_14697378086932.

Head-sharded (tensor-parallel) multi-head attention over 8 NeuronCores:
each core computes 2 of the 16 heads end-to-end.

Per core c (all tiles split by batch so Tile's per-tile dependency
tracking lets batch 1's projections overlap batch 0's attention):
  - phase A1 (serial prefix, DMA-gated): QKV projections for batch 0.
    Q^T/K^T/V^T in [128 local channels, tokens] layout via f32r matmuls
    contracting the hidden dim; K's activation writes straight into the
    zero-padded per-head KzA/KzB tiles; V is written bf16 and transposed
    into per-key-chunk [token, channel] layout by the DMA XBAR (not the
    PE).  rotate_half is a permutation matmul on the PE; cores 1..7 get
    cos=1/sin=0 so rope degenerates to the identity.
  - batch 0 attention blocks run next (flash-style over 128-token key
    chunks: S^T = Kz Q^T, P^T = exp(S^T) on ScalarE -- the phase
    bottleneck -- O^T = [V|1]^T P^T giving the softmax denominator as a
    free 65th PSUM row).  Batch 1's QKV groups, rope, and V transposes
    are injected into the block stream as side work, using the
    oproj/keep-warm PSUM banks that batch-0 blocks don't need yet.
  - batch 1 attention blocks absorb all output projections (normalize =
    fast approx reciprocal + gpsimd broadcast + one PSUM-direct
    multiply; oproj = O_loc @ Wo per 128-token chunk, bf16 partials
    DMA'd out; host sums partials and adds bo).
Throughput notes: ALL matmuls are padded to a full 128-row/column PE
footprint (the clock gate throttles half-array work to 1.2 GHz); the
S/exp/PV software pipeline carries across block boundaries; side work
replaces keep-warm filler wherever possible.
"""
import sys

sys.path.insert(0, "/opt/trn_rl_repo")

import numpy as np
import ml_dtypes

import concourse.bass as bass
import concourse.mybir as mybir
from concourse import bacc
from concourse.bass import ts, ds
from concourse.tile import TileContext
from concourse.masks import make_identity
from concourse.bass_utils import run_bass_kernel_spmd

F32 = mybir.dt.float32
F32R = mybir.dt.float32r
BF16 = mybir.dt.bfloat16

P = 128          # partitions / local channels per core
HID = 1024       # hidden
NT = 4096        # total tokens (batch 2 x 2048)
NB = 2048        # tokens per batch
HD = 64          # head dim
N_CORES = 8

_NC_CACHE = None


def build_nc():
    nc = bacc.Bacc("TRN2")

    # xt: host-rearranged [p, chunk, o, n_tail]: each 512-token chunk is
    # one contiguous 16KB line per partition (fewest DMA descriptors)
    xt = nc.declare_dram_parameter("xt", [P, 8, 8, 512], F32R, isOutput=False)
    wq = nc.declare_dram_parameter("wq", [P, 8, P], F32R, isOutput=False)
    wk = nc.declare_dram_parameter("wk", [P, 8, P], F32R, isOutput=False)
    wv = nc.declare_dram_parameter("wv", [P, 8, P], F32R, isOutput=False)
    wo = nc.declare_dram_parameter("wo", [P, HID], BF16, isOutput=False)
    bia = nc.declare_dram_parameter("bias", [P, 3], F32, isOutput=False)
    cos = nc.declare_dram_parameter("cos", [HD, NT], BF16, isOutput=False)
    sin = nc.declare_dram_parameter("sin", [HD, NT], BF16, isOutput=False)
    rmat = nc.declare_dram_parameter("rmat", [P, P], F32R, isOutput=False)
    out = nc.declare_dram_parameter("out", [NT, HID], BF16, isOutput=True)

    with TileContext(nc) as tc:
        with tc.tile_pool(name="consts", bufs=1) as consts, \
             tc.tile_pool(name="big", bufs=1) as big, \
             tc.tile_pool(name="xtp", bufs=6) as xtp, \
             tc.tile_pool(name="ropet", bufs=2) as ropet, \
             tc.tile_pool(name="ptp", bufs=7) as ptp, \
             tc.tile_pool(name="osb", bufs=3) as osb, \
             tc.tile_pool(name="nrm", bufs=1) as nrm, \
             tc.tile_pool(name="spS", bufs=2, space="PSUM") as spS, \
             tc.tile_pool(name="spO", bufs=1, space="PSUM") as spO, \
             tc.tile_pool(name="spP", bufs=1, space="PSUM") as spP, \
             tc.tile_pool(name="spD", bufs=1, space="PSUM") as spD:

            # ---------------- constants (Activation HWDGE queue, parallel
            # with the xt stream on the SP queue)
            wqs = consts.tile([P, 8, P], F32R)
            wks = consts.tile([P, 8, P], F32R)
            wvs = consts.tile([P, 8, P], F32R)
            nc.scalar.dma_start(wqs, wq[:])
            nc.scalar.dma_start(wks, wk[:])
            nc.scalar.dma_start(wvs, wv[:])
            bias_t = consts.tile([P, 3], F32)
            nc.scalar.dma_start(bias_t, bia[:])
            rmat_t = consts.tile([P, P], F32R)
            nc.scalar.dma_start(rmat_t, rmat[:])
            cos_t = consts.tile([HD, NT], BF16)
            sin_t = consts.tile([HD, NT], BF16)
            nc.scalar.dma_start(cos_t, cos[:])
            nc.scalar.dma_start(sin_t, sin[:])
            wos = consts.tile([P, HID], BF16)
            nc.scalar.dma_start(wos, wo[:])
            ident = consts.tile([P, P], BF16)
            make_identity(nc, ident)

            # ---------------- per-batch state, split into 1024-token
            # half tiles so consumers wait only on the half they read
            # Q^T per (batch, nq-half) [128 ch, 1024 tok]
            Qt = [[big.tile([P, 1024], F32R, name=f"Qt{b}{n}")
                   for n in range(2)] for b in range(2)]
            # zero-padded per-head K^T: head A in rows 0..63, head B in
            # rows 64..127; the K activation writes its halves directly
            # into these (no intermediate Kt tile)
            KzA = [[big.tile([P, 1024], F32R, name=f"KzA{b}{n}")
                    for n in range(2)] for b in range(2)]
            KzB = [[big.tile([P, 1024], F32R, name=f"KzB{b}{n}")
                    for n in range(2)] for b in range(2)]
            for b in range(2):
                for n in range(2):
                    nc.vector.memset(KzA[b][n][HD:P, :].bitcast(F32), 0.0)
                    nc.vector.memset(KzB[b][n][0:HD, :].bitcast(F32), 0.0)
            # V^T staging in bf16, shared across batches (batch 1's
            # projection rewrites it only after batch 0's transposes read it)
            _vtb = big.tile([P, NB], BF16, name="Vtb")
            Vtb = [_vtb, _vtb]
            # V in [token, chunk, 64 V | 1 | 63 zeros] layout per
            # (head, batch, 8-chunk half)
            VaugA = [[big.tile([P, 8, P], BF16, name=f"VaugA{b}{n}")
                      for n in range(2)] for b in range(2)]
            VaugB = [[big.tile([P, 8, P], BF16, name=f"VaugB{b}{n}")
                      for n in range(2)] for b in range(2)]
            for b in range(2):
                for V in (*VaugA[b], *VaugB[b]):
                    nc.vector.memset(V, 0.0)
                    nc.vector.memset(V[:, :, 64:65], 1.0)
            # normalized attention out^T, one tile per 512 tokens so
            # output projections unblock at fine grain
            OtT = [big.tile([P, 512], BF16, name=f"Ot{k}") for k in range(8)]

            # ---------------- building blocks
            def load_chunk(c):
                # one 512-token chunk of x^T as two [128, 4 o-chunks, 512]
                # tiles so the first projection matmuls wait on 1MB, not 2MB
                ta = xtp.tile([P, 4, 512], F32R, tag="xt", name="xta")
                nc.sync.dma_start(ta, xt[:, c, 0:4])
                tb = xtp.tile([P, 4, 512], F32R, tag="xt", name="xtb")
                nc.sync.dma_start(tb, xt[:, c, 4:8])
                return (ta, tb)

            def qkv_pair(xtt0, xtt1, wt, acc, c0):
                # one projection for a pair of 512-token chunks into a
                # 2-bank accumulator; paired matmuls share their stationary
                # so every second one skips LDWEIGHTS
                for o in range(8):
                    nc.tensor.matmul(acc[:, 0:512], wt[:, o],
                                     xtt0[o // 4][:, o % 4],
                                     start=(o == 0), stop=(o == 7))
                    nc.tensor.matmul(acc[:, 512:1024], wt[:, o],
                                     xtt1[o // 4][:, o % 4],
                                     start=(o == 0), stop=(o == 7))

            def act_q(acc, b, l0, ln, dve=False):
                dst = Qt[b][l0 // 1024][:, ds(l0 % 1024, ln)]
                if dve:
                    nc.vector.tensor_scalar_add(dst, acc, bias_t[:, 0:1])
                else:
                    nc.scalar.activation(dst, acc,
                                         mybir.ActivationFunctionType.Identity,
                                         bias=bias_t[:, 0:1])

            def act_k(acc, b, l0, ln, dve=False):
                dA = KzA[b][l0 // 1024][0:HD, ds(l0 % 1024, ln)]
                dB = KzB[b][l0 // 1024][HD:P, ds(l0 % 1024, ln)]
                if dve:
                    nc.vector.tensor_scalar_add(dA, acc[0:HD],
                                                bias_t[0:HD, 1:2])
                    nc.vector.tensor_scalar_add(dB, acc[HD:P],
                                                bias_t[HD:P, 1:2])
                else:
                    nc.scalar.activation(dA, acc[0:HD],
                                         mybir.ActivationFunctionType.Identity,
                                         bias=bias_t[0:HD, 1:2])
                    nc.scalar.activation(dB, acc[HD:P],
                                         mybir.ActivationFunctionType.Identity,
                                         bias=bias_t[HD:P, 1:2])

            def act_v(acc, b, l0, ln, dve=False):
                if dve:
                    nc.vector.tensor_scalar_add(Vtb[b][:, ds(l0, ln)], acc,
                                                bias_t[:, 2:3])
                else:
                    nc.scalar.activation(Vtb[b][:, ds(l0, ln)], acc,
                                         mybir.ActivationFunctionType.Identity,
                                         bias=bias_t[:, 2:3])

            def rope_slice(tiles, b, l0, psr):
                # rope rows 0..63 of the 512-token slice at local offset l0
                t = tiles[b][l0 // 1024]
                lsl = ds(l0 % 1024, 512)
                gsl = ds(b * NB + l0, 512)
                nc.tensor.matmul(psr, rmat_t, t[:, lsl],
                                 start=True, stop=True)
                tmp = ropet.tile([HD, 512], F32, tag="tmp", name="tmp")
                nc.vector.tensor_tensor(tmp, psr[0:HD], sin_t[:, gsl],
                                        mybir.AluOpType.mult)
                nc.vector.tensor_tensor(t[0:HD, lsl], t[0:HD, lsl],
                                        cos_t[:, gsl], mybir.AluOpType.mult)
                nc.vector.tensor_tensor(t[0:HD, lsl], t[0:HD, lsl], tmp,
                                        mybir.AluOpType.add)

            def vtrans_chunk(b, kc, pst_bf, slot):
                # V [ch, tok] -> [tok, ch] per 128-token chunk: one bf16 PE
                # transpose (both heads at once) into a bf16 view of a
                # shared PSUM bank, then two copies into the Vaug layouts
                dst = pst_bf[:, ts(slot, P)]
                nc.tensor.transpose(dst, Vtb[b][:, ts(kc, P)], ident)
                nc.vector.tensor_copy(VaugA[b][kc // 8][:, kc % 8, 0:HD],
                                      dst[:, 0:HD])
                nc.vector.tensor_copy(VaugB[b][kc // 8][:, kc % 8, 0:HD],
                                      dst[:, HD:P])

            # ---------------- phase A1: batch 0 projections (serial prefix)
            xtts = [load_chunk(c) for c in range(2)]
            for pr in range(2):           # chunk pairs (0,1) and (2,3)
                c0 = 2 * pr
                x0, x1 = xtts
                accQ = spS.tile([P, 1024], F32, tag="S", name="accQ")
                qkv_pair(x0, x1, wqs, accQ, c0)
                accK = spS.tile([P, 1024], F32, tag="S", name="accK")
                qkv_pair(x0, x1, wks, accK, c0)
                accV = spO.tile([P, 1024], F32, tag="O", name="accV")
                qkv_pair(x0, x1, wvs, accV, c0)
                # prefetch next pair while activations drain
                if pr == 0:
                    xtts = [load_chunk(c) for c in (2, 3)]
                l0p = (c0 // 2) * 1024
                act_q(accQ, 0, l0p, 1024)
                act_k(accK, 0, l0p, 1024)
                act_v(accV, 0, l0p, 1024)
                pstV = spO.tile([P, 1024], F32, tag="O", name="pstV")
                pst_bf = pstV.bitcast(BF16)
                for u in range(2):
                    l0 = c0 * 512 + u * 512
                    psr = spP.tile([P, 512], F32, tag="oproj", name="psrQ")
                    rope_slice(Qt, 0, l0, psr)
                    psr2 = spD.tile([P, 512], F32, tag="dummy", name="psrK")
                    rope_slice(KzA, 0, l0, psr2)
                    for s in range(4):
                        kc = (c0 + u) * 4 + s
                        vtrans_chunk(0, kc, pst_bf, u * 4 + s)

            # ---------------- batch-1 side work, injected into the batch-0
            # attention stream.  Each item is a closure using spP/spD banks.
            # x chunks are prefetched (the in-order PE queue must never wait
            # on a 2MB transfer started at consumption time).
            side_work = {bi: [] for bi in range(8)}
            a2 = {}
            for c in range(3):
                a2[c] = load_chunk(4 + c)

            def a2_load(c):
                def fn(c=c):
                    a2[c] = load_chunk(4 + c)
                return fn

            def a2_pair(which, c0):
                # one projection pair on spP+spD (both banks), c0 in {0,2}
                def fn(which=which, c0=c0):
                    wt, actf = {"q": (wqs, act_q), "k": (wks, act_k),
                                "v": (wvs, act_v)}[which]
                    acc0 = spP.tile([P, 512], F32, tag="oproj", name="a2a")
                    acc1 = spD.tile([P, 512], F32, tag="dummy", name="a2b")
                    for o in range(8):
                        nc.tensor.matmul(acc0, wt[:, o],
                                         a2[c0][o // 4][:, o % 4],
                                         start=(o == 0), stop=(o == 7))
                        nc.tensor.matmul(acc1, wt[:, o],
                                         a2[c0 + 1][o // 4][:, o % 4],
                                         start=(o == 0), stop=(o == 7))
                    actf(acc0, 1, c0 * 512, 512, dve=True)
                    actf(acc1, 1, c0 * 512 + 512, 512, dve=True)
                return fn

            def a2_rope(l0):
                def fn(l0=l0):
                    psr = spP.tile([P, 512], F32, tag="oproj", name="psrQ2")
                    rope_slice(Qt, 1, l0, psr)
                    psr2 = spD.tile([P, 512], F32, tag="dummy", name="psrK2")
                    rope_slice(KzA, 1, l0, psr2)
                return fn

            def a2_vtrans(u):
                def fn(u=u):
                    pool = spP if u % 2 == 0 else spD
                    tag = "oproj" if u % 2 == 0 else "dummy"
                    pst = pool.tile([P, 512], F32, tag=tag, name="pstV2")
                    pst_bf = pst.bitcast(BF16)
                    for s in range(4):
                        vtrans_chunk(1, u * 4 + s, pst_bf, s)
                return fn

            # (fn, cooldown): cooldown = S/PV slots to leave after the
            # item so its PSUM-bank WAR (the activation read) clears before
            # the next side item re-allocates the same bank
            side_work[0] = [(a2_pair("q", 0), 2), (a2_pair("k", 0), 2)]
            side_work[1] = [(a2_pair("v", 0), 1), (a2_load(3), 0),
                            (a2_rope(0), 1), (a2_rope(512), 1),
                            (a2_vtrans(0), 1), (a2_vtrans(1), 1)]
            side_work[2] = [(a2_pair("q", 2), 2), (a2_pair("k", 2), 2),
                            (a2_rope(1024), 1)]
            side_work[3] = [(a2_pair("v", 2), 1), (a2_rope(1536), 1),
                            (a2_vtrans(2), 1), (a2_vtrans(3), 1)]

            # ---------------- attention + output projection
            def oproj_tile(t0):
                # output projection of one 128-token chunk (both heads);
                # the two halves use different psum banks so the second
                # matmul never queues behind the first half's PSUM read
                lhs = OtT[t0 // 512][:, ts((t0 % 512) // P, P)]
                ost = osb.tile([P, HID], BF16, tag="ost", name="ost")
                Pps = spP.tile([P, 512], F32, tag="oproj", name="opj")
                nc.tensor.matmul(Pps, lhs, wos[:, 0:512],
                                 start=True, stop=True)
                nc.any.tensor_copy(ost[:, 0:512], Pps)
                Pps2 = spD.tile([P, 512], F32, tag="dummy", name="opj2")
                nc.tensor.matmul(Pps2, lhs, wos[:, 512:1024],
                                 start=True, stop=True)
                nc.any.tensor_copy(ost[:, 512:1024], Pps2)
                nc.sync.dma_start(out[t0:t0 + P, :], ost)

            def normalize(hlo, q0, Ops, qlen):
                # custom DVE/gpsimd ops ignore partition offsets, so stage
                # the den row at partition 0 (single cheap copy), then
                # fast approx reciprocal -> gpsimd broadcast -> one
                # multiply straight out of PSUM
                den0 = nrm.tile([1, 1024], F32, tag="den0",
                                name="den0")[:, 0:qlen]
                nc.vector.tensor_copy(den0, Ops[HD:HD + 1, 0:qlen])
                rc = nrm.tile([1, 1024], F32, tag="rc", name="rc")[:, 0:qlen]
                nc.vector.reciprocal_approx_fast(rc, den0)
                rcb = nrm.tile([HD, 1024], F32, tag="rcb",
                               name="rcb")[:, 0:qlen]
                nc.gpsimd.partition_broadcast(rcb, rc)
                for j in range(qlen // 512):
                    nc.vector.tensor_tensor(
                        OtT[q0 // 512 + j][hlo:hlo + HD, :],
                        Ops[0:HD, ts(j, 512)],
                        rcb[:, ts(j, 512)],
                        mybir.AluOpType.mult)

            oproj_queue = []
            blocks = [(b, 1024 * nqb, 1024, h)
                      for b in (0, 1) for nqb in (0, 1) for h in (0, 1)]
            pend = []        # (pv_fn, chunk_idx, Pt) pipeline carry-over
            prev_ctx = None  # (hlo, q0, Ops, qlen, bi) awaiting normalize
            for bi, (b, lq0, qlen, h) in enumerate(blocks):
                q0 = b * NB + lq0
                Vaug = VaugA[b] if h == 0 else VaugB[b]
                Kz = KzA[b] if h == 0 else KzB[b]
                Qb = Qt[b][lq0 // 1024]
                hlo = h * HD

                def s_exp(i, Kz=Kz, Qb=Qb, lq0=lq0, qlen=qlen):
                    Sps = spS.tile([P, 1024], F32, tag="S", name="Sps")
                    for hf in range(qlen // 512):
                        nc.tensor.matmul(
                            Sps[:, ts(hf, 512)],
                            Kz[i // 8][:, ts(i % 8, P)],
                            Qb[:, ds(lq0 % 1024 + hf * 512, 512)],
                            start=True, stop=True)
                    Pt = ptp.tile([P, 1024], BF16, tag="P", name="Pt")
                    nc.scalar.activation(
                        Pt[:, 0:qlen], Sps[:, 0:qlen],
                        mybir.ActivationFunctionType.Exp)
                    return Pt

                # the pipeline carries ACROSS block boundaries: issue this
                # block's first DEPTH S/exp chunks interleaved with the
                # previous block's tail PVs, then its normalize, so neither
                # the PE nor ScalarE drains between blocks
                DEPTH = 6
                sw = side_work.get(bi, [])
                cool = 0
                first_pts = []
                for k in range(DEPTH):
                    first_pts.append(s_exp(k))
                    if pend:
                        f, idx, pt = pend.pop(0)
                        f(idx, pt)
                    if cool > 0:
                        cool -= 1
                    elif sw:
                        fn, cool = sw.pop(0)
                        fn()
                    elif oproj_queue and bi - oproj_queue[0][1] >= 2:
                        oproj_tile(oproj_queue.pop(0)[0])
                if prev_ctx is not None:
                    phlo, pq0, pOps, pqlen, pbi = prev_ctx
                    normalize(phlo, pq0, pOps, pqlen)
                    if phlo:     # both heads of this q-range now normalized
                        for tch in range(pqlen // P):
                            oproj_queue.append((pq0 + tch * P, pbi))

                Ops = spO.tile([P, 1024], F32, tag="O", name="Ops")

                def pv(i, Pt, Vaug=Vaug, Ops=Ops, qlen=qlen):
                    for hf in range(qlen // 512):
                        nc.tensor.matmul(
                            Ops[:, ts(hf, 512)],
                            Vaug[i // 8][:, i % 8, :],
                            Pt[:, ts(hf, 512)],
                            start=(i == 0), stop=(i == 15),
                            skip_group_check=True)

                pend = [(pv, k, first_pts[k]) for k in range(DEPTH)]
                for i in range(DEPTH, 16):
                    pend.append((pv, i, s_exp(i)))
                    f, idx, pt = pend.pop(0)
                    f(idx, pt)
                    min_age = 2 if i < 10 else 1
                    if cool > 0:
                        cool -= 1
                    elif sw:
                        fn, cool = sw.pop(0)
                        fn()
                    elif oproj_queue and bi - oproj_queue[0][1] >= min_age:
                        oproj_tile(oproj_queue.pop(0)[0])
                while sw:
                    fn, cool = sw.pop(0)
                    fn()
                prev_ctx = (hlo, q0, Ops, qlen, bi)

            # drain the last block's pipeline + normalize
            for f, idx, pt in pend:
                f(idx, pt)
            phlo, pq0, pOps, pqlen, pbi = prev_ctx
            normalize(phlo, pq0, pOps, pqlen)
            for tch in range(pqlen // P):
                oproj_queue.append((pq0 + tch * P, pbi))
            # remaining output projections round-robin over four PSUM
            # banks so four matmul->copy chains overlap
            dr2 = spS.tile([P, 1024], F32, tag="S", name="dr2")
            dr3 = spS.tile([P, 1024], F32, tag="S", name="dr3")
            drO = spO.tile([P, 1024], F32, tag="O", name="drO")
            drain_banks = [dr2, dr3, drO]
            for dbi, (t0, _) in enumerate(oproj_queue):
                lhs = OtT[t0 // 512][:, ts((t0 % 512) // P, P)]
                ost = osb.tile([P, HID], BF16, tag="ost", name="ost")
                bank = drain_banks[dbi % 3]
                for hf in range(2):
                    nc.tensor.matmul(bank[:, ts(hf, 512)], lhs,
                                     wos[:, ts(hf, 512)],
                                     start=True, stop=True,
                                     skip_group_check=True)
                if dbi % 2 == 0:
                    nc.scalar.activation(
                        ost, bank, mybir.ActivationFunctionType.Identity)
                else:
                    nc.vector.tensor_copy(ost, bank)
                nc.sync.dma_start(out[t0:t0 + P, :], ost)

    nc.compile()
    return nc


def _get_nc():
    global _NC_CACHE
    if _NC_CACHE is None:
        _NC_CACHE = build_nc()
    return _NC_CACHE


def shard_inputs(x, rope_cos, rope_sin, Wq, bq, Wk, bk, Wv, bv, Wo, bo):
    """Build per-core input maps."""
    # [p, chunk, o, n_tail]: per partition, one contiguous 16KB chunk line
    xt = np.ascontiguousarray(
        x.reshape(NT, HID).T.reshape(8, P, 8, 512).transpose(1, 2, 0, 3)
    ).astype(np.float32)
    cosT = np.ascontiguousarray(rope_cos.reshape(NT, HD).T).astype(np.float32)
    sinT = np.ascontiguousarray(rope_sin.reshape(NT, HD).T).astype(np.float32)
    cos_id = np.ones((HD, NT), np.float32)
    sin_id = np.zeros((HD, NT), np.float32)
    # rotate_half as matrix R: out = R @ t, R[2i,2i+1]=-1, R[2i+1,2i]=+1.
    # matmul computes lhsT.T @ rhs, so pass R.T.
    R = np.zeros((P, P), np.float32)
    idx = np.arange(0, HD, 2)
    R[idx, idx + 1] = -1.0
    R[idx + 1, idx] = 1.0
    rmat = np.ascontiguousarray(R.T)

    in_maps = []
    for c in range(N_CORES):
        lo, hi = c * P, (c + 1) * P
        in_maps.append({
            "xt": xt,
            "wq": np.ascontiguousarray(
                Wq[:, lo:hi].reshape(8, P, P).transpose(1, 0, 2)
            ).astype(np.float32),
            "wk": np.ascontiguousarray(
                Wk[:, lo:hi].reshape(8, P, P).transpose(1, 0, 2)
            ).astype(np.float32),
            "wv": np.ascontiguousarray(
                Wv[:, lo:hi].reshape(8, P, P).transpose(1, 0, 2)
            ).astype(np.float32),
            "wo": np.ascontiguousarray(Wo[lo:hi, :]).astype(ml_dtypes.bfloat16),
            "bias": np.ascontiguousarray(
                np.stack([bq[lo:hi], bk[lo:hi], bv[lo:hi]], axis=1)
            ).astype(np.float32),
            "cos": (cosT if c == 0 else cos_id).astype(ml_dtypes.bfloat16),
            "sin": (sinT if c == 0 else sin_id).astype(ml_dtypes.bfloat16),
            "rmat": rmat,
        })
    return in_maps


def run_device(inputs, trace=False, **kw):
    nc = _get_nc()
    in_maps = shard_inputs(**inputs)
    res = run_bass_kernel_spmd(nc, in_maps, core_ids=list(range(N_CORES)),
                               trace=trace, **kw)
    return res


def gather(res, bo):
    acc = res.results[0]["out"].astype(np.float32)
    for c in range(1, N_CORES):
        acc = acc + res.results[c]["out"].astype(np.float32)
    acc += bo[None, :].astype(np.float32)
    return acc.reshape(2, NB, HID)


def kernel(**inputs):
    # NRT_EXEC_UNIT_UNRECOVERABLE crashes are transient on this fleet;
    # one retry rescues the run.
    try:
        res = run_device(inputs, trace=False)
    except Exception:
        res = run_device(inputs, trace=False)
    return gather(res, np.asarray(inputs["bo"], np.float32))
